# revision 20
# baseline (speedup 1.0000x reference)
"""Shifted abs-diff cost volume kernel for Trainium2 (8 NeuronCores).

out[n, d, y, x] = |image1[n,0,y,x] - image2[n,0,y,x-d]|  (0 where x < d)

The problem is output-DMA bound: 490 MB out vs 7.7 MB in. HBM write
bandwidth per NeuronCore sustains ~400 GB/s aggregate (~197 GB/s per
DMA queue); the f32 baseline ran at that roofline (172.7us). This
version computes and stores the output in bf16 (tolerance is 2e-2;
bf16 introduces ~4e-3), halving the output bytes -> ~75us DMA floor.
The host converts back to f32.

Sharding: pure data parallel over flattened (N*H) rows -> 96 rows/core.

Per-core layout: each row is split into 4 column quarters of SEG=312;
the 96x4 = 384 quarter-segments are packed onto all 128 SBUF
partitions (3 "slots" per partition, partition p = 32*quarter +
(row%32), slot s = row//32).

bf16 DVE perf modes require every operand's innermost step to be +-1
on a 4-byte-aligned base, so the disparity axis (which shifts the img2
read window by one element per d) is split by parity: img2 is stored
twice per slot, once shifted for even d (E, pad 128) and once for odd
d (O, pad 127), and each group of G=8 disparities is computed by two
grouped TensorTensor subtracts with disparity stride -2 (-4 bytes).
All region offsets/strides are even, so the 2x_1p TT mode engages.

abs is split by column so each engine stays under the DMA period: ACT
(1x rate) takes the head ABS_SPLIT columns of each 312-column block,
DVE the tail via a 4x-mode uint16 bitwise-and that clears the sign
bit. The x<d wedge (a static triangular mask, 5% of the output) is
zeroed by the host during unshard; the device ships |img1| there.

Each group's output is shipped as three per-slot DMAs (0.64 MB each)
round-robined over the three available queues - SP HWDGE, GPSIMD
SWDGE, ACT HWDGE - so the aggregate stream is not limited by the
~197 GB/s per-queue rate and the end-of-kernel drain is one slot
chunk, not a whole group.
"""

import ml_dtypes
import numpy as np

import concourse.bass as bass
import concourse.tile as tile
from concourse import mybir
from concourse.ap import AP
from concourse.bass_utils import run_bass_kernel_spmd

N, C, H, W = 2, 1, 384, 1248
D = 128  # MAXDISP
NCORES = 8
ROWS = (N * H) // NCORES  # 96 rows per core
Q = 4  # column quarters per row
SEG = W // Q  # 312 columns per segment
SLOTS = ROWS * Q // 128  # 3 segments per partition
GROUP = 8  # base group size; mid-stream groups fuse two (16)
LE = 440  # img2 region length (both parities), even
A_OFF, E_OFF, O_OFF = 0, SEG, SEG + LE  # 0, 312, 752
SLOT_COLS = SEG + 2 * LE  # 1192
IN_COLS = SLOTS * SLOT_COLS  # 3576
E_PAD, O_PAD = 128, 127  # left zero-pad of the two img2 copies
ABS_SPLIT = {8: 200, 16: 192}  # head cols/block -> ACT; tail -> DVE
# (d0, gsize, per-slot ramp): one small ramp group, 16-disparity
# mid-stream groups (halves per-instruction fixed costs), 8-disparity
# trailing groups (finer end-of-kernel DMA granularity).
SCHEDULE = [(0, 8, True)] + [(8 + 16 * k, 16, False) for k in range(6)] + [
    (104, 8, False), (112, 8, False), (120, 8, False)]
WSKIP = 56  # max wedge columns skipped in quarter-0 output DMAs
TFREE = SLOTS * 16 * SEG  # 14976 free elements per (largest) work tile
BF16 = mybir.dt.bfloat16
F32 = mybir.dt.float32
NPBF16 = ml_dtypes.bfloat16

_NC_CACHE = {}


def build_program():
    nc = bass.Bass("TRN2", target_bir_lowering=False, debug=False)
    imgs_d = nc.dram_tensor("images", [128, IN_COLS], BF16, kind="ExternalInput").ap()
    # Per-core output layout [slot, partition, d, x-in-segment]; the
    # host reassembles rows/quarters. Slot DMAs write [s, p, d0:d0+G, :]
    # as 4992-byte contiguous chunks per (s, p).
    out_d = nc.dram_tensor("out", [SLOTS, 128, D * SEG], BF16, kind="ExternalOutput").ap()

    with tile.TileContext(nc) as tc:
        with (
            tc.tile_pool(name="inp", bufs=1) as inp_pool,
            tc.tile_pool(name="work", bufs=6) as work_pool,
        ):
            # Warm the ACT Abs table set off the critical path (the
            # first real abs otherwise eats a ~1.3us ACT_TABLE_LOAD).
            warm = inp_pool.tile([128, 2], BF16)
            nc.vector.memset(warm[:, :], 1.0)
            nc.scalar.activation(
                warm[:, :], warm[:, :], mybir.ActivationFunctionType.Abs
            )

            # Input loaded per slot block so the first per-slot TT waits
            # on only a third of the input bytes.
            imgs = inp_pool.tile([128, IN_COLS], BF16)
            for s in range(SLOTS):
                nc.sync.dma_start(
                    out=imgs[:, s * SLOT_COLS : (s + 1) * SLOT_COLS],
                    in_=imgs_d[:, s * SLOT_COLS : (s + 1) * SLOT_COLS],
                )

            # Slot-DMAs rotate evenly over the three issuers (SP
            # HWDGE, GPSIMD SWDGE, ACT HWDGE): the 16 shared SDMA
            # engines round-robin queues at packet granularity, so one
            # queue sustains only ~200 GB/s of the ~400 GB/s aggregate
            # -- every queue must stay under that fair share. The
            # quarter-0 partitions' DMAs skip min(d0, WSKIP) leading
            # wedge columns (never written; host masks x<d), saving
            # ~3.5% of output bytes while descriptors stay >= 512 B.
            main_engs = [nc.sync, nc.gpsimd, nc.scalar]
            q0_engs = [nc.gpsimd, nc.sync]
            main_n = 0
            q0_n = 0
            for d0, gsize, ramp in SCHEDULE:
                hg = gsize // 2
                c0 = ABS_SPLIT[gsize]
                t = work_pool.tile([128, TFREE], BF16, tag="t")

                # slot s = None: whole-tile ops; otherwise per-slot.
                slots = list(range(SLOTS)) if ramp else [None]
                for s in slots:
                    nslots = 1 if s is not None else SLOTS
                    sbase = 0 if s is None else s * gsize * SEG
                    ibase = 0 if s is None else s * SLOT_COLS

                    # t[p, s, i, x] = img1[p,s,x] - img2{E,O}[p,s,j],
                    # j = x + pad - (d0 + i); one TT per disparity
                    # parity so the i stride is -2 (4B, keeps 2x mode).
                    for par, roff, pad in ((0, E_OFF, E_PAD), (1, O_OFF, O_PAD)):
                        out_ap = AP(
                            t.tensor,
                            sbase + par * SEG,
                            [
                                [TFREE, 128],
                                [gsize * SEG, nslots],
                                [2 * SEG, hg],
                                [1, SEG],
                            ],
                        )
                        in0 = AP(
                            imgs.tensor,
                            ibase + A_OFF,
                            [[IN_COLS, 128], [SLOT_COLS, nslots], [0, hg], [1, SEG]],
                        )
                        in1 = AP(
                            imgs.tensor,
                            ibase + roff + pad - d0 - par,
                            [[IN_COLS, 128], [SLOT_COLS, nslots], [-2, hg], [1, SEG]],
                        )
                        nc.vector.tensor_sub(out_ap, in0, in1)

                    # abs, split by column: ACT takes the head, DVE
                    # the tail at 4x. The x<d wedge columns carry
                    # |img1| junk; the host masks them to zero.
                    act_ap = AP(
                        t.tensor,
                        sbase,
                        [
                            [TFREE, 128],
                            [gsize * SEG, nslots],
                            [SEG, gsize],
                            [1, c0],
                        ],
                    )
                    nc.scalar.activation(
                        act_ap, act_ap, mybir.ActivationFunctionType.Abs
                    )
                    # abs = clear the bf16 sign bit: bitwise AND with
                    # 0x7FFF on a uint16 view (single-src -> 4x mode;
                    # abs_max isn't a valid tensor_scalar ALU op here).
                    dve_ap = AP(
                        t.tensor,
                        sbase + c0,
                        [
                            [TFREE, 128],
                            [gsize * SEG, nslots],
                            [SEG, gsize],
                            [1, SEG - c0],
                        ],
                    ).bitcast(mybir.dt.uint16)
                    nc.vector.tensor_scalar(
                        dve_ap, dve_ap, 0x7FFF, None, mybir.AluOpType.bitwise_and
                    )

                # DMA out per slot on rotating queues. DRAM dims
                # (p, cols) match the tile's (partition, cols) walk.
                skip = min(d0, WSKIP)
                for so in range(SLOTS):
                    obase = so * 128 * D * SEG + d0 * SEG
                    tbase = so * gsize * SEG
                    if skip == 0:
                        dram_ap = AP(
                            out_d.tensor, obase,
                            [[D * SEG, 128], [1, gsize * SEG]],
                        )
                        sbuf_ap = AP(
                            t.tensor, tbase,
                            [[TFREE, 128], [1, gsize * SEG]],
                        )
                        main_engs[main_n % 3].dma_start(out=dram_ap, in_=sbuf_ap)
                        main_n += 1
                        continue
                    # partitions 32-127 (quarters 1-3): full blocks
                    dram_ap = AP(
                        out_d.tensor, obase + 32 * D * SEG,
                        [[D * SEG, 96], [1, gsize * SEG]],
                    )
                    sbuf_ap = AP(
                        t.tensor, 32 * TFREE + tbase,
                        [[TFREE, 96], [1, gsize * SEG]],
                    )
                    main_engs[main_n % 3].dma_start(out=dram_ap, in_=sbuf_ap)
                    main_n += 1
                    # partitions 0-31 (quarter 0): skip leading wedge
                    dram_ap = AP(
                        out_d.tensor, obase + skip,
                        [[D * SEG, 32], [SEG, gsize], [1, SEG - skip]],
                    )
                    sbuf_ap = AP(
                        t.tensor, tbase + skip,
                        [[TFREE, 32], [SEG, gsize], [1, SEG - skip]],
                    )
                    q0_engs[q0_n % 2].dma_start(out=dram_ap, in_=sbuf_ap)
                    q0_n += 1
    return nc


def split_excess_waits(nc):
    """Split multi-wait instructions for this walrus build's ISA encoder.

    The TRN2 ISA encoding here holds 1 semaphore wait per engine
    instruction (2 for a standalone EventSemaphore). Tile's scheduler
    fuses up to ~3 waits per instruction, which this neuronxcc rejects
    with "Too many sync wait commands". Moving the excess waits into
    EventSemaphore instructions issued just before, on the same engine
    queue, is semantically identical (the engine stalls at the sync
    instruction instead).
    """
    counter = 0
    for f in nc.m.functions:
        for b in f.blocks:
            plan = []  # (index, [event_insts]) in original order
            insts = b.instructions
            for idx, inst in enumerate(insts):
                si = inst.sync_info
                if si is None:
                    continue
                waits = list(si.on_wait)
                cap = 2 if inst.opcode == "EventSemaphore" else 1
                if len(waits) <= cap:
                    continue
                extra, keep = waits[:-cap], waits[-cap:]
                evs = []
                for j in range(0, len(extra), 2):
                    ev = mybir.InstEventSemaphore(
                        name=f"EVWS-{counter}",
                        opcode="EventSemaphore",
                        engine=inst.engine,
                    )
                    counter += 1
                    ev.sync_info = mybir.SyncInfo(
                        on_wait=extra[j : j + 2], on_update=[]
                    )
                    evs.append(ev)
                inst.sync_info = mybir.SyncInfo(
                    on_wait=keep, on_update=list(si.on_update)
                )
                plan.append((idx, evs))
            # apply inserts back-to-front so earlier indices stay valid
            for idx, evs in reversed(plan):
                for k, ev in enumerate(evs):
                    insts.insert(idx + k, ev)
    return nc


def get_program():
    if "nc" not in _NC_CACHE:
        _NC_CACHE["nc"] = split_excess_waits(build_program())
    return _NC_CACHE["nc"]


def shard_inputs(image1, image2):
    img1 = np.asarray(image1, dtype=np.float32).reshape(N * H, W).astype(NPBF16)
    img2 = np.asarray(image2, dtype=np.float32).reshape(N * H, W).astype(NPBF16)
    nh = N * H
    z = lambda k: np.zeros((nh, k), NPBF16)
    # E copy: value[j] = img2[col + j - 128]; O copy: ... + j - 127.
    # Right-pad so the j-gather below stays in bounds for the last
    # quarter (indices up to 3*SEG + LE - 1).
    img2E = np.concatenate([z(E_PAD), img2, z(1)], axis=1)
    img2O = np.concatenate([z(O_PAD), img2, z(2)], axis=1)
    p = np.arange(128)
    c, rm = p // 32, p % 32
    jA = np.arange(SEG)
    jR = np.arange(LE)
    maps = []
    for k in range(NCORES):
        rows = slice(k * ROWS, (k + 1) * ROWS)
        i1, i2E, i2O = img1[rows], img2E[rows], img2O[rows]
        packed = np.empty((128, IN_COLS), NPBF16)
        for s in range(SLOTS):
            r = (32 * s + rm)[:, None]  # [128,1]
            base = s * SLOT_COLS
            cs = c[:, None] * SEG
            packed[:, base + A_OFF : base + A_OFF + SEG] = i1[r, cs + jA]
            packed[:, base + E_OFF : base + E_OFF + LE] = i2E[r, cs + jR]
            packed[:, base + O_OFF : base + O_OFF + LE] = i2O[r, cs + jR]
        maps.append({"images": np.ascontiguousarray(packed)})
    return maps


def unshard_output(per_core):
    out = np.empty((N, D * C, H, W), dtype=np.float32)
    for k in range(NCORES):
        n = (k * ROWS) // H
        y0 = (k * ROWS) % H
        a = np.asarray(per_core[k]).astype(np.float32)
        a = a.reshape(SLOTS, 4, 32, D, SEG)  # s, c, rm, d, x
        # rows r = 32*s + rm ; cols = c*SEG + x
        blk = a.transpose(3, 0, 2, 1, 4).reshape(D, ROWS, W)  # d, (s,rm), (c,x)
        out[n, :, y0 : y0 + ROWS, :] = blk
    # out[:, d, :, x] = 0 for x < d by construction (the shifted read
    # falls off the left image edge) -- a static mask the device
    # doesn't spend write bandwidth on.
    for d in range(1, D):
        out[:, d, :, :d] = 0.0
    return out


def kernel(image1, image2):
    nc = get_program()
    res = run_bass_kernel_spmd(nc, shard_inputs(image1, image2), list(range(NCORES)))
    return unshard_output([res.results[k]["out"] for k in range(NCORES)])


# revision 21
# speedup vs baseline: 1.2596x; 1.2596x over previous
"""Shifted abs-diff cost volume kernel for Trainium2 (8 NeuronCores).

out[n, d, y, x] = |image1[n,0,y,x] - image2[n,0,y,x-d]|  (0 where x < d)

The problem is output-DMA bound: 490 MB out vs 7.7 MB in. HBM write
bandwidth per NeuronCore sustains ~400 GB/s aggregate (~197 GB/s per
DMA queue); the f32 baseline ran at that roofline (172.7us). This
version computes and stores the output in bf16 (tolerance is 2e-2;
bf16 introduces ~4e-3), halving the output bytes -> ~75us DMA floor.
The host converts back to f32.

Sharding: pure data parallel over flattened (N*H) rows -> 96 rows/core.

Per-core layout: each row is split into 4 column quarters of SEG=312;
the 96x4 = 384 quarter-segments are packed onto all 128 SBUF
partitions (3 "slots" per partition, partition p = 32*quarter +
(row%32), slot s = row//32).

bf16 DVE perf modes require every operand's innermost step to be +-1
on a 4-byte-aligned base, so the disparity axis (which shifts the img2
read window by one element per d) is split by parity: img2 is stored
twice per slot, once shifted for even d (E, pad 128) and once for odd
d (O, pad 127), and each group of G=8 disparities is computed by two
grouped TensorTensor subtracts with disparity stride -2 (-4 bytes).
All region offsets/strides are even, so the 2x_1p TT mode engages.

abs is split by column so each engine stays under the DMA period: ACT
(1x rate) takes the head ABS_SPLIT columns of each 312-column block,
DVE the tail via a 4x-mode uint16 bitwise-and that clears the sign
bit. The x<d wedge (a static triangular mask, 5% of the output) is
zeroed by the host during unshard; the device ships |img1| there.

Each group's output is shipped as three per-slot DMAs (0.64 MB each)
round-robined over the three available queues - SP HWDGE, GPSIMD
SWDGE, ACT HWDGE - so the aggregate stream is not limited by the
~197 GB/s per-queue rate and the end-of-kernel drain is one slot
chunk, not a whole group.
"""

import ml_dtypes
import numpy as np

import concourse.bass as bass
import concourse.tile as tile
from concourse import mybir
from concourse.ap import AP
from concourse.bass_utils import run_bass_kernel_spmd

N, C, H, W = 2, 1, 384, 1248
D = 128  # MAXDISP
NCORES = 8
ROWS = (N * H) // NCORES  # 96 rows per core
Q = 4  # column quarters per row
SEG = W // Q  # 312 columns per segment
SLOTS = ROWS * Q // 128  # 3 segments per partition
GROUP = 8  # base group size; mid-stream groups fuse two (16)
LE = 440  # img2 region length (both parities), even
A_OFF, E_OFF, O_OFF = 0, SEG, SEG + LE  # 0, 312, 752
SLOT_COLS = SEG + 2 * LE  # 1192
IN_COLS = SLOTS * SLOT_COLS  # 3576
E_PAD, O_PAD = 128, 127  # left zero-pad of the two img2 copies
ABS_SPLIT = {8: 200, 16: 192}  # head cols/block -> ACT; tail -> DVE
# (d0, gsize, per-slot ramp): one small ramp group, 16-disparity
# mid-stream groups (halves per-instruction fixed costs), 8-disparity
# trailing groups (finer end-of-kernel DMA granularity).
SCHEDULE = [(0, 8, True)] + [(8 + 16 * k, 16, False) for k in range(6)] + [
    (104, 8, False), (112, 8, False), (120, 8, False)]
TFREE = SLOTS * 16 * SEG  # 14976 free elements per (largest) work tile
BF16 = mybir.dt.bfloat16
F32 = mybir.dt.float32
NPBF16 = ml_dtypes.bfloat16

_NC_CACHE = {}


def build_program():
    nc = bass.Bass("TRN2", target_bir_lowering=False, debug=False)
    imgs_d = nc.dram_tensor("images", [128, IN_COLS], BF16, kind="ExternalInput").ap()
    # Per-core output layout [slot, partition, d, x-in-segment]; the
    # host reassembles rows/quarters. Slot DMAs write [s, p, d0:d0+G, :]
    # as 4992-byte contiguous chunks per (s, p).
    out_d = nc.dram_tensor("out", [SLOTS, 128, D * SEG], BF16, kind="ExternalOutput").ap()

    with tile.TileContext(nc) as tc:
        with (
            tc.tile_pool(name="inp", bufs=1) as inp_pool,
            tc.tile_pool(name="work", bufs=6) as work_pool,
        ):
            # Warm the ACT Abs table set off the critical path (the
            # first real abs otherwise eats a ~1.3us ACT_TABLE_LOAD).
            warm = inp_pool.tile([128, 2], BF16)
            nc.vector.memset(warm[:, :], 1.0)
            nc.scalar.activation(
                warm[:, :], warm[:, :], mybir.ActivationFunctionType.Abs
            )

            # Input loaded per slot block so the first per-slot TT waits
            # on only a third of the input bytes.
            imgs = inp_pool.tile([128, IN_COLS], BF16)
            for s in range(SLOTS):
                nc.sync.dma_start(
                    out=imgs[:, s * SLOT_COLS : (s + 1) * SLOT_COLS],
                    in_=imgs_d[:, s * SLOT_COLS : (s + 1) * SLOT_COLS],
                )

            # Slot-DMAs rotate evenly over the three issuers (SP
            # HWDGE, GPSIMD SWDGE, ACT HWDGE): the 16 shared SDMA
            # engines round-robin queues at packet granularity, so one
            # queue sustains only ~200 GB/s of the ~400 GB/s aggregate
            # -- every queue must stay under that fair share. The
            main_engs = [nc.sync, nc.gpsimd, nc.scalar]
            main_n = 0
            for d0, gsize, ramp in SCHEDULE:
                hg = gsize // 2
                c0 = ABS_SPLIT[gsize]
                t = work_pool.tile([128, TFREE], BF16, tag="t")

                # slot s = None: whole-tile ops; otherwise per-slot.
                slots = list(range(SLOTS)) if ramp else [None]
                for s in slots:
                    nslots = 1 if s is not None else SLOTS
                    sbase = 0 if s is None else s * gsize * SEG
                    ibase = 0 if s is None else s * SLOT_COLS

                    # t[p, s, i, x] = img1[p,s,x] - img2{E,O}[p,s,j],
                    # j = x + pad - (d0 + i); one TT per disparity
                    # parity so the i stride is -2 (4B, keeps 2x mode).
                    for par, roff, pad in ((0, E_OFF, E_PAD), (1, O_OFF, O_PAD)):
                        out_ap = AP(
                            t.tensor,
                            sbase + par * SEG,
                            [
                                [TFREE, 128],
                                [gsize * SEG, nslots],
                                [2 * SEG, hg],
                                [1, SEG],
                            ],
                        )
                        in0 = AP(
                            imgs.tensor,
                            ibase + A_OFF,
                            [[IN_COLS, 128], [SLOT_COLS, nslots], [0, hg], [1, SEG]],
                        )
                        in1 = AP(
                            imgs.tensor,
                            ibase + roff + pad - d0 - par,
                            [[IN_COLS, 128], [SLOT_COLS, nslots], [-2, hg], [1, SEG]],
                        )
                        nc.vector.tensor_sub(out_ap, in0, in1)

                    # abs, split by column: ACT takes the head, DVE
                    # the tail at 4x. The x<d wedge columns carry
                    # |img1| junk; the host masks them to zero.
                    act_ap = AP(
                        t.tensor,
                        sbase,
                        [
                            [TFREE, 128],
                            [gsize * SEG, nslots],
                            [SEG, gsize],
                            [1, c0],
                        ],
                    )
                    nc.scalar.activation(
                        act_ap, act_ap, mybir.ActivationFunctionType.Abs
                    )
                    # abs = clear the bf16 sign bit: bitwise AND with
                    # 0x7FFF on a uint16 view (single-src -> 4x mode;
                    # abs_max isn't a valid tensor_scalar ALU op here).
                    dve_ap = AP(
                        t.tensor,
                        sbase + c0,
                        [
                            [TFREE, 128],
                            [gsize * SEG, nslots],
                            [SEG, gsize],
                            [1, SEG - c0],
                        ],
                    ).bitcast(mybir.dt.uint16)
                    nc.vector.tensor_scalar(
                        dve_ap, dve_ap, 0x7FFF, None, mybir.AluOpType.bitwise_and
                    )

                # DMA out, one per slot on rotating queues. DRAM dims
                # (p, cols) match the tile's (partition, cols) walk.
                for so in range(SLOTS):
                    dram_ap = AP(
                        out_d.tensor,
                        so * 128 * D * SEG + d0 * SEG,
                        [[D * SEG, 128], [1, gsize * SEG]],
                    )
                    sbuf_ap = AP(
                        t.tensor,
                        so * gsize * SEG,
                        [[TFREE, 128], [1, gsize * SEG]],
                    )
                    main_engs[main_n % 3].dma_start(out=dram_ap, in_=sbuf_ap)
                    main_n += 1
    return nc


def split_excess_waits(nc):
    """Split multi-wait instructions for this walrus build's ISA encoder.

    The TRN2 ISA encoding here holds 1 semaphore wait per engine
    instruction (2 for a standalone EventSemaphore). Tile's scheduler
    fuses up to ~3 waits per instruction, which this neuronxcc rejects
    with "Too many sync wait commands". Moving the excess waits into
    EventSemaphore instructions issued just before, on the same engine
    queue, is semantically identical (the engine stalls at the sync
    instruction instead).
    """
    counter = 0
    for f in nc.m.functions:
        for b in f.blocks:
            plan = []  # (index, [event_insts]) in original order
            insts = b.instructions
            for idx, inst in enumerate(insts):
                si = inst.sync_info
                if si is None:
                    continue
                waits = list(si.on_wait)
                cap = 2 if inst.opcode == "EventSemaphore" else 1
                if len(waits) <= cap:
                    continue
                extra, keep = waits[:-cap], waits[-cap:]
                evs = []
                for j in range(0, len(extra), 2):
                    ev = mybir.InstEventSemaphore(
                        name=f"EVWS-{counter}",
                        opcode="EventSemaphore",
                        engine=inst.engine,
                    )
                    counter += 1
                    ev.sync_info = mybir.SyncInfo(
                        on_wait=extra[j : j + 2], on_update=[]
                    )
                    evs.append(ev)
                inst.sync_info = mybir.SyncInfo(
                    on_wait=keep, on_update=list(si.on_update)
                )
                plan.append((idx, evs))
            # apply inserts back-to-front so earlier indices stay valid
            for idx, evs in reversed(plan):
                for k, ev in enumerate(evs):
                    insts.insert(idx + k, ev)
    return nc


def get_program():
    if "nc" not in _NC_CACHE:
        _NC_CACHE["nc"] = split_excess_waits(build_program())
    return _NC_CACHE["nc"]


def shard_inputs(image1, image2):
    img1 = np.asarray(image1, dtype=np.float32).reshape(N * H, W).astype(NPBF16)
    img2 = np.asarray(image2, dtype=np.float32).reshape(N * H, W).astype(NPBF16)
    nh = N * H
    z = lambda k: np.zeros((nh, k), NPBF16)
    # E copy: value[j] = img2[col + j - 128]; O copy: ... + j - 127.
    # Right-pad so the j-gather below stays in bounds for the last
    # quarter (indices up to 3*SEG + LE - 1).
    img2E = np.concatenate([z(E_PAD), img2, z(1)], axis=1)
    img2O = np.concatenate([z(O_PAD), img2, z(2)], axis=1)
    p = np.arange(128)
    c, rm = p // 32, p % 32
    jA = np.arange(SEG)
    jR = np.arange(LE)
    maps = []
    for k in range(NCORES):
        rows = slice(k * ROWS, (k + 1) * ROWS)
        i1, i2E, i2O = img1[rows], img2E[rows], img2O[rows]
        packed = np.empty((128, IN_COLS), NPBF16)
        for s in range(SLOTS):
            r = (32 * s + rm)[:, None]  # [128,1]
            base = s * SLOT_COLS
            cs = c[:, None] * SEG
            packed[:, base + A_OFF : base + A_OFF + SEG] = i1[r, cs + jA]
            packed[:, base + E_OFF : base + E_OFF + LE] = i2E[r, cs + jR]
            packed[:, base + O_OFF : base + O_OFF + LE] = i2O[r, cs + jR]
        maps.append({"images": np.ascontiguousarray(packed)})
    return maps


def unshard_output(per_core):
    out = np.empty((N, D * C, H, W), dtype=np.float32)
    for k in range(NCORES):
        n = (k * ROWS) // H
        y0 = (k * ROWS) % H
        a = np.asarray(per_core[k]).astype(np.float32)
        a = a.reshape(SLOTS, 4, 32, D, SEG)  # s, c, rm, d, x
        # rows r = 32*s + rm ; cols = c*SEG + x
        blk = a.transpose(3, 0, 2, 1, 4).reshape(D, ROWS, W)  # d, (s,rm), (c,x)
        out[n, :, y0 : y0 + ROWS, :] = blk
    # out[:, d, :, x] = 0 for x < d by construction (the shifted read
    # falls off the left image edge) -- a static mask the device
    # doesn't spend write bandwidth on.
    for d in range(1, D):
        out[:, d, :, :d] = 0.0
    return out


def kernel(image1, image2):
    nc = get_program()
    res = run_bass_kernel_spmd(nc, shard_inputs(image1, image2), list(range(NCORES)))
    return unshard_output([res.results[k]["out"] for k in range(NCORES)])


# revision 22
# speedup vs baseline: 1.5045x; 1.1944x over previous
"""Shifted abs-diff cost volume kernel for Trainium2 (8 NeuronCores).

out[n, d, y, x] = |image1[n,0,y,x] - image2[n,0,y,x-d]|  (0 where x < d)

The problem is output-DMA bound: 490 MB out vs 7.7 MB in. HBM write
bandwidth per NeuronCore sustains ~400 GB/s aggregate (~197 GB/s per
DMA queue); the f32 baseline ran at that roofline (172.7us). This
version computes and stores the output in bf16 (tolerance is 2e-2;
bf16 introduces ~4e-3), halving the output bytes -> ~75us DMA floor.
The host converts back to f32.

Sharding: pure data parallel over flattened (N*H) rows -> 96 rows/core.

Per-core layout: each row is split into 4 column quarters of SEG=312;
the 96x4 = 384 quarter-segments are packed onto all 128 SBUF
partitions (3 "slots" per partition, partition p = 32*quarter +
(row%32), slot s = row//32).

bf16 DVE perf modes require every operand's innermost step to be +-1
on a 4-byte-aligned base, so the disparity axis (which shifts the img2
read window by one element per d) is split by parity: img2 is stored
twice per slot, once shifted for even d (E, pad 128) and once for odd
d (O, pad 127), and each group of G=8 disparities is computed by two
grouped TensorTensor subtracts with disparity stride -2 (-4 bytes).
All region offsets/strides are even, so the 2x_1p TT mode engages.

abs is split by column so each engine stays under the DMA period: ACT
(1x rate) takes the head ABS_SPLIT columns of each 312-column block,
DVE the tail via a 4x-mode uint16 bitwise-and that clears the sign
bit. The x<d wedge (a static triangular mask, 5% of the output) is
zeroed by the host during unshard; the device ships |img1| there.

Each group's output is shipped as three per-slot DMAs (0.64 MB each)
round-robined over the three available queues - SP HWDGE, GPSIMD
SWDGE, ACT HWDGE - so the aggregate stream is not limited by the
~197 GB/s per-queue rate and the end-of-kernel drain is one slot
chunk, not a whole group.
"""

import ml_dtypes
import numpy as np

import concourse.bass as bass
import concourse.tile as tile
from concourse import mybir
from concourse.ap import AP
from concourse.bass_utils import run_bass_kernel_spmd

N, C, H, W = 2, 1, 384, 1248
D = 128  # MAXDISP
NCORES = 8
ROWS = (N * H) // NCORES  # 96 rows per core
Q = 4  # column quarters per row
SEG = W // Q  # 312 columns per segment
SLOTS = ROWS * Q // 128  # 3 segments per partition
GROUP = 8  # disparities per group (two parity TTs of 4 each)
NGROUPS = D // GROUP
HGRP = GROUP // 2  # disparities per parity TT
LE = 440  # img2 region length (both parities), even
A_OFF, E_OFF, O_OFF = 0, SEG, SEG + LE  # 0, 312, 752
SLOT_COLS = SEG + 2 * LE  # 1192
IN_COLS = SLOTS * SLOT_COLS  # 3576
E_PAD, O_PAD = 128, 127  # left zero-pad of the two img2 copies
ABS_SPLIT = 200  # head columns of each SEG block -> ACT; tail -> DVE
RAMP_GROUPS = 1  # leading groups processed per-slot to shorten the ramp
TFREE = SLOTS * GROUP * SEG  # 7488 free elements per work tile
BF16 = mybir.dt.bfloat16
F32 = mybir.dt.float32
NPBF16 = ml_dtypes.bfloat16

_NC_CACHE = {}


def build_program():
    nc = bass.Bass("TRN2", target_bir_lowering=False, debug=False)
    imgs_d = nc.dram_tensor("images", [128, IN_COLS], BF16, kind="ExternalInput").ap()
    # Per-core output layout [slot, partition, d, x-in-segment]; the
    # host reassembles rows/quarters. Slot DMAs write [s, p, d0:d0+G, :]
    # as 4992-byte contiguous chunks per (s, p).
    out_d = nc.dram_tensor("out", [SLOTS, 128, D * SEG], BF16, kind="ExternalOutput").ap()

    with tile.TileContext(nc) as tc:
        with (
            tc.tile_pool(name="inp", bufs=1) as inp_pool,
            tc.tile_pool(name="work", bufs=8) as work_pool,
        ):
            # Warm the ACT Abs table set off the critical path (the
            # first real abs otherwise eats a ~1.3us ACT_TABLE_LOAD).
            warm = inp_pool.tile([128, 2], BF16)
            nc.vector.memset(warm[:, :], 1.0)
            nc.scalar.activation(
                warm[:, :], warm[:, :], mybir.ActivationFunctionType.Abs
            )

            # Input loaded per slot block so the first per-slot TT waits
            # on only a third of the input bytes.
            imgs = inp_pool.tile([128, IN_COLS], BF16)
            for s in range(SLOTS):
                nc.sync.dma_start(
                    out=imgs[:, s * SLOT_COLS : (s + 1) * SLOT_COLS],
                    in_=imgs_d[:, s * SLOT_COLS : (s + 1) * SLOT_COLS],
                )

            dma_engs = [nc.sync, nc.gpsimd, nc.scalar]
            dma_n = 0
            for g in range(NGROUPS):
                d0 = g * GROUP
                t = work_pool.tile([128, TFREE], BF16, tag="t")

                # slot s = None: whole-tile ops; otherwise per-slot.
                slots = list(range(SLOTS)) if g < RAMP_GROUPS else [None]
                for s in slots:
                    nslots = 1 if s is not None else SLOTS
                    sbase = 0 if s is None else s * GROUP * SEG
                    ibase = 0 if s is None else s * SLOT_COLS

                    # t[p, s, i, x] = img1[p,s,x] - img2{E,O}[p,s,j],
                    # j = x + pad - (d0 + i); one TT per disparity
                    # parity so the i stride is -2 (4B, keeps 2x mode).
                    for par, roff, pad in ((0, E_OFF, E_PAD), (1, O_OFF, O_PAD)):
                        out_ap = AP(
                            t.tensor,
                            sbase + par * SEG,
                            [
                                [TFREE, 128],
                                [GROUP * SEG, nslots],
                                [2 * SEG, HGRP],
                                [1, SEG],
                            ],
                        )
                        in0 = AP(
                            imgs.tensor,
                            ibase + A_OFF,
                            [[IN_COLS, 128], [SLOT_COLS, nslots], [0, HGRP], [1, SEG]],
                        )
                        in1 = AP(
                            imgs.tensor,
                            ibase + roff + pad - d0 - par,
                            [[IN_COLS, 128], [SLOT_COLS, nslots], [-2, HGRP], [1, SEG]],
                        )
                        nc.vector.tensor_sub(out_ap, in0, in1)

                    # abs, split by column: ACT takes the head, DVE
                    # the tail at 4x. The x<d wedge columns carry
                    # |img1| junk; the host masks them to zero.
                    act_ap = AP(
                        t.tensor,
                        sbase,
                        [
                            [TFREE, 128],
                            [GROUP * SEG, nslots],
                            [SEG, GROUP],
                            [1, ABS_SPLIT],
                        ],
                    )
                    nc.scalar.activation(
                        act_ap, act_ap, mybir.ActivationFunctionType.Abs
                    )
                    # abs = clear the bf16 sign bit: bitwise AND with
                    # 0x7FFF on a uint16 view (single-src -> 4x mode;
                    # abs_max isn't a valid tensor_scalar ALU op here).
                    dve_ap = AP(
                        t.tensor,
                        sbase + ABS_SPLIT,
                        [
                            [TFREE, 128],
                            [GROUP * SEG, nslots],
                            [SEG, GROUP],
                            [1, SEG - ABS_SPLIT],
                        ],
                    ).bitcast(mybir.dt.uint16)
                    nc.vector.tensor_scalar(
                        dve_ap, dve_ap, 0x7FFF, None, mybir.AluOpType.bitwise_and
                    )

                # DMA out, one per slot on rotating queues. DRAM dims
                # (p, cols) match the tile's (partition, cols) walk.
                for so in range(SLOTS):
                    dram_ap = AP(
                        out_d.tensor,
                        so * 128 * D * SEG + d0 * SEG,
                        [[D * SEG, 128], [1, GROUP * SEG]],
                    )
                    sbuf_ap = AP(
                        t.tensor,
                        so * GROUP * SEG,
                        [[TFREE, 128], [1, GROUP * SEG]],
                    )
                    dma_engs[dma_n % 3].dma_start(out=dram_ap, in_=sbuf_ap)
                    dma_n += 1
    return nc


def split_excess_waits(nc):
    """Split multi-wait instructions for this walrus build's ISA encoder.

    The TRN2 ISA encoding here holds 1 semaphore wait per engine
    instruction (2 for a standalone EventSemaphore). Tile's scheduler
    fuses up to ~3 waits per instruction, which this neuronxcc rejects
    with "Too many sync wait commands". Moving the excess waits into
    EventSemaphore instructions issued just before, on the same engine
    queue, is semantically identical (the engine stalls at the sync
    instruction instead).
    """
    counter = 0
    for f in nc.m.functions:
        for b in f.blocks:
            plan = []  # (index, [event_insts]) in original order
            insts = b.instructions
            for idx, inst in enumerate(insts):
                si = inst.sync_info
                if si is None:
                    continue
                waits = list(si.on_wait)
                cap = 2 if inst.opcode == "EventSemaphore" else 1
                if len(waits) <= cap:
                    continue
                extra, keep = waits[:-cap], waits[-cap:]
                evs = []
                for j in range(0, len(extra), 2):
                    ev = mybir.InstEventSemaphore(
                        name=f"EVWS-{counter}",
                        opcode="EventSemaphore",
                        engine=inst.engine,
                    )
                    counter += 1
                    ev.sync_info = mybir.SyncInfo(
                        on_wait=extra[j : j + 2], on_update=[]
                    )
                    evs.append(ev)
                inst.sync_info = mybir.SyncInfo(
                    on_wait=keep, on_update=list(si.on_update)
                )
                plan.append((idx, evs))
            # apply inserts back-to-front so earlier indices stay valid
            for idx, evs in reversed(plan):
                for k, ev in enumerate(evs):
                    insts.insert(idx + k, ev)
    return nc


def get_program():
    if "nc" not in _NC_CACHE:
        _NC_CACHE["nc"] = split_excess_waits(build_program())
    return _NC_CACHE["nc"]


def shard_inputs(image1, image2):
    img1 = np.asarray(image1, dtype=np.float32).reshape(N * H, W).astype(NPBF16)
    img2 = np.asarray(image2, dtype=np.float32).reshape(N * H, W).astype(NPBF16)
    nh = N * H
    z = lambda k: np.zeros((nh, k), NPBF16)
    # E copy: value[j] = img2[col + j - 128]; O copy: ... + j - 127.
    # Right-pad so the j-gather below stays in bounds for the last
    # quarter (indices up to 3*SEG + LE - 1).
    img2E = np.concatenate([z(E_PAD), img2, z(1)], axis=1)
    img2O = np.concatenate([z(O_PAD), img2, z(2)], axis=1)
    p = np.arange(128)
    c, rm = p // 32, p % 32
    jA = np.arange(SEG)
    jR = np.arange(LE)
    maps = []
    for k in range(NCORES):
        rows = slice(k * ROWS, (k + 1) * ROWS)
        i1, i2E, i2O = img1[rows], img2E[rows], img2O[rows]
        packed = np.empty((128, IN_COLS), NPBF16)
        for s in range(SLOTS):
            r = (32 * s + rm)[:, None]  # [128,1]
            base = s * SLOT_COLS
            cs = c[:, None] * SEG
            packed[:, base + A_OFF : base + A_OFF + SEG] = i1[r, cs + jA]
            packed[:, base + E_OFF : base + E_OFF + LE] = i2E[r, cs + jR]
            packed[:, base + O_OFF : base + O_OFF + LE] = i2O[r, cs + jR]
        maps.append({"images": np.ascontiguousarray(packed)})
    return maps


def unshard_output(per_core):
    out = np.empty((N, D * C, H, W), dtype=np.float32)
    for k in range(NCORES):
        n = (k * ROWS) // H
        y0 = (k * ROWS) % H
        a = np.asarray(per_core[k]).astype(np.float32)
        a = a.reshape(SLOTS, 4, 32, D, SEG)  # s, c, rm, d, x
        # rows r = 32*s + rm ; cols = c*SEG + x
        blk = a.transpose(3, 0, 2, 1, 4).reshape(D, ROWS, W)  # d, (s,rm), (c,x)
        out[n, :, y0 : y0 + ROWS, :] = blk
    # out[:, d, :, x] = 0 for x < d by construction (the shifted read
    # falls off the left image edge) -- a static mask the device
    # doesn't spend write bandwidth on.
    for d in range(1, D):
        out[:, d, :, :d] = 0.0
    return out


def kernel(image1, image2):
    nc = get_program()
    res = run_bass_kernel_spmd(nc, shard_inputs(image1, image2), list(range(NCORES)))
    return unshard_output([res.results[k]["out"] for k in range(NCORES)])
